# revision 1
# baseline (speedup 1.0000x reference)
"""Farthest Point Sampling (FPS) Trainium2 Bass kernel.

Problem: x (8, 3, 65536) fp32 -> y (8, 3, 512) fp32 where
y[b, :, j] = x[b, :, idx[b, j]] and idx = farthest-point-sampling indices
(PointNet++ style, start at index 0, M=512 points).

Sharding: batch-parallel, one point cloud per NeuronCore (8 cores).

Per-core algorithm (N = 65536 points laid out as 128 partitions x 512,
flat index n = p*512 + f):
  dist = +BIG
  c = pts[0]; emit c
  repeat 511 times:
    sq_c   = (coord_c - c_c)^2           (ACT Square, scale=-1, bias=c_c)
    d      = (sqx + sqy) + sqz           (DVE adds; matches jnp sum order)
    dist   = min(dist, d); pmax = rowmax (fused DVE tensor_tensor_reduce)
    gmax   = max over partitions          (GPSIMD partition_all_reduce max)
    m      = (dist == gmax)               (one-hot mask; DVE tensor_scalar)
    w_c    = sum_f(m * coord_c) per row   (DVE scalar_tensor_tensor accum)
    c      = sum over partitions of w     (GPSIMD partition_all_reduce add)
    emit c
All arithmetic is bit-exact with the jax reference (verified vs numpy
prototype), so the selected index trajectory matches exactly.
"""

import sys

import numpy as np

for _p in ("/opt/trn_rl_repo", "/root/.axon_site/_ro/trn_rl_repo"):
    if _p not in sys.path:
        sys.path.append(_p)

import concourse.bacc as bacc
import concourse.mybir as mybir
from concourse.bass_isa import ReduceOp
from concourse.bass_utils import run_bass_kernel_spmd
from concourse.tile import TileContext

B, C, N = 8, 3, 65536
M = 512
P, F = 128, 512  # SBUF layout: n = p*F + f
FP32 = mybir.dt.float32
BIG = 3.0e38


def build_nc(m_steps: int = M):
    nc = bacc.Bacc(None, target_bir_lowering=False, debug=False)
    x = nc.declare_dram_parameter("x", [C, N], FP32, isOutput=False)
    y = nc.declare_dram_parameter("y", [C, m_steps], FP32, isOutput=True)

    with TileContext(nc) as tc:
        with tc.tile_pool(name="state", bufs=1) as state:
            p3 = state.tile([P, 3 * F], FP32)      # [xs | ys | zs]
            distA = state.tile([P, F], FP32)
            distB = state.tile([P, F], FP32)
            sq = state.tile([P, 3 * F], FP32)      # x-, z-squares (ACT)
            sqy = state.tile([P, F], FP32)         # y-square (DVE)
            dy = state.tile([P, F], FP32)
            a1 = state.tile([P, F], FP32)
            d = state.tile([P, F], FP32)
            junk = state.tile([P, F], FP32)
            w = state.tile([P, 3], FP32)           # per-partition candidates
            wg = state.tile([P, 3], FP32)          # winner-gated candidates
            pm = state.tile([P, 1], FP32)
            gmaxb = state.tile([P, 1], FP32)
            rowS = state.tile([1, 3], FP32)
            bias = state.tile([P, 3], FP32)        # current centroid, bcast
            out_int = state.tile([1, 3 * m_steps], FP32)

            # ---- load points: x[c] (65536,) -> p3[:, c*F:(c+1)*F]
            for c in range(C):
                nc.sync.dma_start(
                    out=p3[:, c * F:(c + 1) * F],
                    in_=x[c:c + 1, :].rearrange("o (p f) -> (o p) f", p=P),
                )
            nc.vector.memset(distA, BIG)

            # ---- step 0: centroid = point 0
            nc.sync.dma_start(out=rowS, in_=x[:, 0:1].rearrange("c o -> o c"))
            nc.vector.tensor_copy(out_int[:, 0:3], rowS)
            # broadcast rowS to all partitions: zeros + row0 -> all-reduce add
            nc.vector.memset(w, 0.0)
            nc.vector.tensor_copy(w[0:1, :], rowS)
            nc.gpsimd.partition_all_reduce(bias, w, P, ReduceOp.add)

            dist_src, dist_dst = distA, distB
            for j in range(1, m_steps):
                # x-, z-squares on ACT: sq_c = Square(-coord + c_c)
                for c in (0, 2):
                    nc.scalar.activation(
                        sq[:, c * F:(c + 1) * F],
                        p3[:, c * F:(c + 1) * F],
                        mybir.ActivationFunctionType.Square,
                        bias=bias[:, c:c + 1],
                        scale=-1.0,
                    )
                # y-square on DVE (runs concurrently with ACT)
                nc.vector.tensor_scalar(
                    out=dy, in0=p3[:, F:2 * F], scalar1=bias[:, 1:2],
                    scalar2=None, op0=mybir.AluOpType.subtract)
                nc.vector.scalar_tensor_tensor(
                    out=sqy, in0=dy, scalar=1.0, in1=dy,
                    op0=mybir.AluOpType.mult, op1=mybir.AluOpType.mult)
                nc.vector.tensor_add(a1, sq[:, 0:F], sqy)
                nc.vector.tensor_add(d, a1, sq[:, 2 * F:3 * F])
                # dist_dst = min(dist_src, d); pm = rowmax(dist_dst)
                nc.vector.tensor_tensor(
                    dist_dst, dist_src, d, op=mybir.AluOpType.min)
                nc.vector.reduce_max(pm, dist_dst, axis=mybir.AxisListType.X)
                # global max on Pool, concurrent with the DVE extraction below
                nc.gpsimd.partition_all_reduce(gmaxb, pm, P, ReduceOp.max)
                # per-partition candidate coords:
                # w[:, c] = sum_f (dist == pm) * coord_c
                for c in range(C):
                    nc.vector.scalar_tensor_tensor(
                        out=junk, in0=dist_dst, scalar=pm[:, 0:1],
                        in1=p3[:, c * F:(c + 1) * F],
                        op0=mybir.AluOpType.is_equal, op1=mybir.AluOpType.mult,
                        accum_out=w[:, c:c + 1],
                    )
                # gate to the winning partition: wg = (pm == gmax) * w
                nc.vector.scalar_tensor_tensor(
                    out=wg, in0=pm[:, 0:1].to_broadcast([P, 3]),
                    scalar=gmaxb[:, 0:1], in1=w,
                    op0=mybir.AluOpType.is_equal, op1=mybir.AluOpType.mult)
                # collapse partitions: every partition gets the winner coords
                nc.gpsimd.partition_all_reduce(bias, wg, P, ReduceOp.add)
                nc.gpsimd.tensor_copy(out_int[:, 3 * j:3 * j + 3], bias[0:1, :])
                dist_src, dist_dst = dist_dst, dist_src

            # ---- write back: y[c, j] = out_int[0, 3j + c]
            for c in range(C):
                nc.sync.dma_start(
                    out=y[c:c + 1, :],
                    in_=out_int.rearrange("o (j c) -> o c j", c=3)[:, c, :],
                )
    return nc


_NC_CACHE = None


def _get_nc():
    global _NC_CACHE
    if _NC_CACHE is None:
        nc = build_nc()
        nc.finalize()
        _NC_CACHE = nc
    return _NC_CACHE


def kernel(x: np.ndarray) -> np.ndarray:
    x = np.ascontiguousarray(np.asarray(x, dtype=np.float32))
    assert x.shape == (B, C, N), x.shape
    nc = _get_nc()
    in_maps = [{"x": x[b]} for b in range(B)]
    res = run_bass_kernel_spmd(nc, in_maps, list(range(B)))
    y = np.stack([np.asarray(res.results[b]["y"]) for b in range(B)])
    return y.astype(np.float32)


if __name__ == "__main__":
    x = np.random.randn(B, C, N).astype(np.float32)
    y = kernel(x)
    print("kernel ran, y shape:", y.shape)

